# revision 22
# baseline (speedup 1.0000x reference)
"""Two-layer GAT on 8 Trainium2 NeuronCores — single launch, Bass/Tile.

v2 design
---------
* Table row order = dst-core-major: node n -> row = core(n)*12544 + pos(n).
  Each core's transform shard IS its own dst block, so per-dst "own row"
  data (h, a_src·h, a_dst·h) stays in SBUF — no reserved gather slots.
* Gather groups are row PHASES (row % 4) via elem_step=512 strided
  dma_gather (int16 idx = row//4 < 25088). Each node's phase is chosen by
  a greedy balancer so every dst segment has near-equal per-phase counts,
  cutting slot padding from ~2.1x to ~1.25x.
* Self-loop edges are folded in analytically from the SBUF-resident own
  rows (never gathered).
* x@W runs sharded (1/8 nodes per core); AllGather broadcasts the table;
  pad rows (positions 12500..12543 of each block) are poisoned with
  a_src = -60000 so their exp() contribution is exactly 0.
* Both layers run in ONE device launch; layer-2 table = elu(out1)@W2p is
  transposed+transformed on-device, AllGathered, and the SAME index blob
  drives both edge phases (identical graph layout).
* Gathers: 1024-idx calls, single_packet=False, 4 SWDGE queues (measured
  ~81 GB/s/core vs 42 GB/s for the default config).
"""
import sys
sys.path.insert(0, "/opt/trn_rl_repo")

import numpy as np

P = 128
NCORE = 8
N = 100000
SH = 12500            # real dst nodes per core
NT = 98
PC = NT * P           # 12544 padded positions per core
NV = PC * NCORE       # 100352 table rows
G = 4                 # phases
PH = NV // G          # 25088 rows per phase (int16-addressable)
F_IN = 512
HH1, CC1 = 8, 8
CT1 = HH1 * CC1       # 64
RW1 = CT1 + 2 * HH1   # 80: [h(64) | as(8) | ad(8)]
OUT2 = 40
RW2 = OUT2 + 2        # 42: [g(40) | as2 | ad2]
NEG = -60000.0
DCH = 16              # slot-columns per gather call (2048 idxs)
PAD_IDX = 3125        # row 12500+g = 4*3125+g  (core-0 pad rows, any phase)

RUNLOG = []           # filled when BASS_TRACE is set (test harness only)


# ==========================================================================
# host-side graph prep
# ==========================================================================

def assign_phases(es, ed, core_id):
    """Greedy phase assignment balancing per-dst-segment phase counts.

    Returns phase[N] int8 with exactly SH//G nodes per (core, phase).
    """
    out_deg = np.bincount(es, minlength=N)
    order = np.argsort(es, kind="stable")
    ed_s = ed[order]
    starts = np.zeros(N + 1, np.int64)
    np.cumsum(out_deg, out=starts[1:])

    cnt = np.zeros((N, G), np.int32)        # per dst, per phase in-counts
    cap = np.full((NCORE, G), SH // G, np.int32)
    phase = np.zeros(N, np.int8)
    BIG = np.int32(1 << 30)
    gidx = np.arange(G)

    proc = np.argsort(-out_deg, kind="stable")
    for s in proc:
        dsts = ed_s[starts[s]:starts[s + 1]]
        r = core_id[s]
        sc = cnt[dsts].sum(axis=0, dtype=np.int64)
        sc = np.where(cap[r] > 0, sc, BIG)
        g = int(np.argmin(sc))
        phase[s] = g
        cap[r, g] -= 1
        cnt[dsts, g] += 1

    for _ in range(3):                       # refinement sweeps
        moved = 0
        for s in proc:
            dsts = ed_s[starts[s]:starts[s + 1]]
            if len(dsts) == 0:
                continue
            r, g0 = core_id[s], phase[s]
            sc = cnt[dsts].sum(axis=0, dtype=np.int64)
            sc[g0] -= len(dsts)
            scm = np.where((cap[r] > 0) | (gidx == g0), sc, BIG)
            g = int(np.argmin(scm))
            if g != g0:
                phase[s] = g
                cap[r, g] -= 1
                cap[r, g0] += 1
                cnt[dsts, g0] -= 1
                cnt[dsts, g] += 1
                moved += 1
        if moved == 0:
            break
    return phase, cnt


def prep_graph(src, dst):
    """Build phases, positions, per-tile slot tables and idx blobs."""
    src = np.asarray(src, np.int64)
    dst = np.asarray(dst, np.int64)
    m = src != dst
    es, ed = src[m], dst[m]

    # dst -> core: deal by in-degree so every core sees the same profile
    indeg = np.bincount(ed, minlength=N)
    by_deg = np.argsort(-indeg, kind="stable")
    core_id = np.empty(N, np.int32)
    core_id[by_deg] = np.arange(N) % NCORE

    phase, cnt = assign_phases(es, ed, core_id)

    # positions: per core, per phase, in-degree-descending fill
    pos = np.empty(N, np.int64)
    orows = np.full((NCORE, PC), -1, np.int64)
    for r in range(NCORE):
        nodes = np.where(core_id == r)[0]
        nodes = nodes[np.argsort(-indeg[nodes], kind="stable")]
        for q in range(G):
            nq = nodes[phase[nodes] == q]
            pq = 4 * np.arange(len(nq)) + q
            pos[nq] = pq
            orows[r, pq] = nq
    row_of = core_id.astype(np.int64) * PC + pos      # table row of node

    # per-core tile max counts -> uniform D table
    D_core = np.zeros((NCORE, NT, G), np.int64)
    for r in range(NCORE):
        carr = np.zeros((PC, G), np.int32)
        nodes = np.where(core_id == r)[0]
        carr[pos[nodes]] = cnt[nodes]
        D_core[r] = carr.reshape(NT, P, G).max(axis=1)
    D_tbl = D_core.max(axis=0)                        # [NT, G]

    tile_off = []
    off = 0
    for t in range(NT):
        tile_off.append(off)
        off += 8 * int(D_tbl[t].sum())
    CB = off

    # per-core idx blobs
    idxval = (row_of[es] // G).astype(np.int16)
    gval = phase[es].astype(np.int64)
    blobs = []
    dmax = int(D_tbl.max())
    for r in range(NCORE):
        mm = core_id[ed] == r
        e_pos = pos[ed[mm]]
        e_g = gval[mm]
        e_idx = idxval[mm]
        key = e_pos * G + e_g
        o = np.argsort(key, kind="stable")
        ks, vs = key[o], e_idx[o]
        cnt_pg = np.bincount(ks, minlength=PC * G)
        st = np.zeros(PC * G, np.int64)
        np.cumsum(cnt_pg[:-1], out=st[1:])
        col = np.arange(len(ks)) - st[ks]
        big = np.full((PC, G, dmax), PAD_IDX, np.int16)
        big[ks // G, ks % G, col] = vs

        blob = np.empty((P, CB), np.int16)
        for t in range(NT):
            c = tile_off[t]
            for g in range(G):
                D = int(D_tbl[t, g])
                mat = big[t * P:(t + 1) * P, g, :D]       # [128, D]
                for ck in range(0, D, DCH):
                    d = min(DCH, D - ck)
                    L = mat[:, ck:ck + d].T.ravel()       # i = col*128+part
                    W16 = L.reshape(-1, 16).T             # [16, 8d]
                    blob[:, c:c + 8 * d] = np.tile(W16, (8, 1))
                    c += 8 * d
        blobs.append(blob)

    return pos, core_id, orows, D_tbl, tile_off, CB, blobs


# ==========================================================================
# device program (single launch, both layers)
# ==========================================================================

def build_launch(D_tbl, tile_off, CB):
    from concourse import mybir
    import concourse.bass as bass
    import concourse.bacc as bacc
    import concourse.tile as tile
    import contextlib

    f16, f32, i16 = mybir.dt.float16, mybir.dt.float32, mybir.dt.int16
    AS1, AD1 = CT1, CT1 + HH1            # 64, 72
    AS2, AD2 = OUT2, OUT2 + 1            # 40, 41

    nc = bacc.Bacc("TRN2", target_bir_lowering=False, debug=False,
                   num_swdge_queues=4, num_devices=8)
    fT = nc.dram_tensor("fT", [F_IN, PC], f16, kind="ExternalInput")
    lnm = nc.dram_tensor("lnm", [P, NT], f32, kind="ExternalInput")
    Wp = nc.dram_tensor("Wp", [F_IN, RW1], f16, kind="ExternalInput")
    W2p = nc.dram_tensor("W2p", [CT1, RW2], f16, kind="ExternalInput")
    b1t = nc.dram_tensor("b1t", [CT1], f32, kind="ExternalInput")
    b2t = nc.dram_tensor("b2t", [OUT2], f32, kind="ExternalInput")
    ident = nc.dram_tensor("ident", [P, P], f16, kind="ExternalInput")
    idxb = nc.dram_tensor("idxb", [P, CB], i16, kind="ExternalInput")
    outo = nc.dram_tensor("out", [PC, OUT2], f32, kind="ExternalOutput")

    cc1 = nc.dram_tensor("cc1", [PC, 128], f16)
    cc2 = nc.dram_tensor("cc2", [PC, 128], f16)
    tb1 = nc.dram_tensor("tb1", [NV, 128], f16, addr_space="Shared")
    tb2 = nc.dram_tensor("tb2", [NV, 128], f16, addr_space="Shared")

    qn = [0]

    def nextq():
        qn[0] = (qn[0] + 1) % 4
        return qn[0]

    def bcast_load(dram_ap, w, pool, dt, tag):
        t = pool.tile([P, w], dt, tag=tag)
        ap = bass.AP(tensor=dram_ap.tensor, offset=dram_ap.offset,
                     ap=[[0, P]] + list(dram_ap.ap))
        nc.sync.dma_start(out=t[:], in_=ap)
        return t

    with tile.TileContext(nc) as tc:
        with contextlib.ExitStack() as ctx:
            singles = ctx.enter_context(tc.tile_pool(name="singles", bufs=1))
            xtp = ctx.enter_context(tc.tile_pool(name="xt", bufs=2))
            psp = ctx.enter_context(tc.tile_pool(name="ps", bufs=4, space="PSUM"))
            ps2 = ctx.enter_context(tc.tile_pool(name="ps2", bufs=2, space="PSUM"))
            gp = ctx.enter_context(tc.tile_pool(name="gp", bufs=4))
            ip = ctx.enter_context(tc.tile_pool(name="ip", bufs=3))
            ep = ctx.enter_context(tc.tile_pool(name="ep", bufs=3))
            sp = ctx.enter_context(tc.tile_pool(name="sp", bufs=4))
            mp = ctx.enter_context(tc.tile_pool(name="mp", bufs=3))
            tp2 = ctx.enter_context(tc.tile_pool(name="tp2", bufs=3))

            # ---------------- singles ----------------
            wts = []
            for kc in range(4):
                wt = singles.tile([P, RW1], f16, tag=f"w{kc}")
                nc.sync.dma_start(out=wt[:], in_=Wp[kc * P:(kc + 1) * P, :])
                wts.append(wt)
            w2sb = singles.tile([CT1, RW2], f16)
            nc.sync.dma_start(out=w2sb[:], in_=W2p[:])
            idt = singles.tile([P, P], f16)
            nc.sync.dma_start(out=idt[:], in_=ident[:])
            bt1 = bcast_load(b1t[:], CT1, singles, f32, "bt1")
            bt2 = bcast_load(b2t[:], OUT2, singles, f32, "bt2")
            # poison rows: zeros, NEG at 40:42 and 64:80
            pois = singles.tile([PC - SH, 128], f16, tag="pois")
            nc.vector.memset(pois[:], 0.0)
            nc.vector.memset(pois[:, AS2:AD2 + 1], NEG)
            nc.vector.memset(pois[:, AS1:AD1 + HH1], NEG)
            # own-row tables (SBUF-resident)
            hsb = singles.tile([P, NT, RW1], f16, tag="hsb")
            h2sb = singles.tile([P, NT, CT1], f16, tag="h2sb")
            gsb = singles.tile([P, NT, RW2], f16, tag="gsb")
            lnmt = singles.tile([P, NT], f32, tag="lnm")
            nc.sync.dma_start(out=lnmt[:], in_=lnm[:])

            # ---------------- transform 1: x@W1p for own block ----------
            chunks = [(c, min(512, PC - c)) for c in range(0, PC, 512)]
            for c0, cw in chunks:
                xts = []
                for kc in range(4):
                    xt = xtp.tile([P, 512], f16, tag=f"x{kc}")
                    nc.sync.dma_start(out=xt[:, :cw],
                                      in_=fT[kc * P:(kc + 1) * P, c0:c0 + cw])
                    xts.append(xt)
                for sub in range(cw // P):
                    ti = (c0 + sub * P) // P
                    pt = psp.tile([P, RW1], f32)
                    for kc in range(4):
                        nc.tensor.matmul(out=pt[:],
                                         lhsT=xts[kc][:, sub * P:(sub + 1) * P],
                                         rhs=wts[kc][:],
                                         start=(kc == 0), stop=(kc == 3))
                    nc.vector.tensor_copy(out=hsb[:, ti, :], in_=pt[:])
                    nc.sync.dma_start(out=cc1[ti * P:(ti + 1) * P, 0:RW1],
                                      in_=hsb[:, ti, :])

            nc.gpsimd.collective_compute(
                "AllGather", mybir.AluOpType.bypass,
                replica_groups=[list(range(NCORE))],
                ins=[cc1[:]], outs=[tb1[:]])
            tc.strict_bb_all_engine_barrier()
            for r in range(NCORE):
                nc.sync.dma_start(out=tb1[r * PC + SH:r * PC + SH + 44, :],
                                  in_=pois[:])
            tc.strict_bb_all_engine_barrier()

            # ---------------- shared edge-phase body --------------------
            def edge_tile(t, tbl, HH, CC, CT, RW, AS, AD, own, bt, is_l1):
                mybir_ = mybir
                Ds = [int(D_tbl[t, g]) for g in range(G)]
                SD = sum(Ds)
                SD1 = SD + 1                      # +1 self slot (computed)
                tcols = 8 * SD
                it = ip.tile([P, tcols], i16, tag="idx")
                nc.sync.dma_start(
                    out=it[:], in_=idxb[:, tile_off[t]:tile_off[t] + tcols])
                Gt = gp.tile([P, SD1, 128], f16, tag="G")
                c0 = 0
                ic = 0
                for g in range(G):
                    D = Ds[g]
                    nck = (D + DCH - 1) // DCH
                    base, rem = (D // nck, D % nck) if nck else (0, 0)
                    for j in range(nck):
                        d = base + (1 if j < rem else 0)
                        ap = bass.AP(tensor=tbl[:].tensor, offset=g * 128,
                                     ap=[[512, PH], [1, 128]])
                        nc.gpsimd.dma_gather(
                            out_ap=Gt[:, c0:c0 + d, :],
                            in_ap=ap,
                            idxs_ap=it[:, ic:ic + 8 * d],
                            num_idxs=P * d, num_idxs_reg=P * d,
                            elem_size=128, elem_step=512,
                            single_packet=False,
                            queue_num=nextq())
                        c0 += d
                        ic += 8 * d
                # self slot: own row (h | as | ad)
                nc.vector.tensor_copy(out=Gt[:, SD, 0:RW], in_=own[:])

                # e = leaky(as_src + ad_own)   [P, SD1, HH] f32
                et = ep.tile([P, SD1, HH], f32, tag="e")
                nc.vector.tensor_tensor(
                    out=et[:],
                    in0=Gt[:, :, AS:AS + HH],
                    in1=own[:, AD:AD + HH].unsqueeze(1)
                        .broadcast_to([P, SD1, HH]),
                    op=mybir_.AluOpType.add)
                ef = et[:].rearrange("p d h -> p (d h)")
                nc.scalar.activation(out=ef, in_=ef,
                                     func=mybir_.ActivationFunctionType.Prelu,
                                     alpha=0.2)
                # self-loop multiplicity: e_self += ln(mult)
                nc.vector.tensor_scalar_add(out=et[:, SD, :],
                                            in0=et[:, SD, :],
                                            scalar1=lnmt[:, t:t + 1])
                # -max, subtract, exp
                nmt = sp.tile([P, HH], f32, tag="nm")
                nc.vector.reduce_max(out=nmt[:],
                                     in_=et[:].rearrange("p d h -> p h d"),
                                     axis=mybir_.AxisListType.X, negate=True)
                nc.vector.tensor_tensor(
                    out=et[:], in0=et[:],
                    in1=nmt[:].unsqueeze(1).broadcast_to([P, SD1, HH]),
                    op=mybir_.AluOpType.add)
                ext = ep.tile([P, SD1, HH], f16, tag="ex")
                nc.scalar.activation(out=ext[:].rearrange("p d h -> p (d h)"),
                                     in_=ef,
                                     func=mybir_.ActivationFunctionType.Exp)

                # denom + reciprocal
                dent = sp.tile([P, HH], f32, tag="den")
                nc.vector.reduce_sum(out=dent[:],
                                     in_=ext[:].rearrange("p d h -> p h d"),
                                     axis=mybir_.AxisListType.X)
                rdt = sp.tile([P, HH], f32, tag="rd")
                nc.vector.reciprocal(out=rdt[:], in_=dent[:])

                # weighted message sum (f16 products, f32 accumulate)
                mg = mp.tile([P, SD1, CT], f16, tag="msg")
                nc.vector.tensor_tensor(
                    out=mg[:].rearrange("p d (h c) -> p d h c", h=HH),
                    in0=Gt[:, :, 0:CT].rearrange("p d (h c) -> p d h c", h=HH),
                    in1=ext[:].unsqueeze(3).broadcast_to([P, SD1, HH, CC]),
                    op=mybir_.AluOpType.mult)
                ort = sp.tile([P, CT], f32, tag="or")
                nc.vector.reduce_sum(out=ort[:],
                                     in_=mg[:].rearrange("p d f -> p f d"),
                                     axis=mybir_.AxisListType.X)

                # normalize + bias
                o1 = sp.tile([P, CT], f32, tag="o1")
                nc.vector.tensor_tensor(
                    out=o1[:].rearrange("p (h c) -> p h c", h=HH),
                    in0=ort[:].rearrange("p (h c) -> p h c", h=HH),
                    in1=rdt[:].unsqueeze(2).broadcast_to([P, HH, CC]),
                    op=mybir_.AluOpType.mult)
                nc.vector.tensor_add(out=o1[:], in0=o1[:], in1=bt[:])

                if is_l1:
                    # elu(x) = relu(x) + exp(min(x,0)) - 1
                    t1 = sp.tile([P, CT], f32, tag="t1")
                    nc.vector.tensor_scalar_min(out=t1[:], in0=o1[:], scalar1=0.0)
                    nc.scalar.activation(out=t1[:], in_=t1[:],
                                         func=mybir_.ActivationFunctionType.Exp)
                    t2 = sp.tile([P, CT], f32, tag="t2")
                    nc.scalar.activation(out=t2[:], in_=o1[:],
                                         func=mybir_.ActivationFunctionType.Relu)
                    nc.vector.tensor_add(out=t1[:], in0=t1[:], in1=t2[:])
                    nc.vector.tensor_scalar_add(out=h2sb[:, t, :], in0=t1[:],
                                                scalar1=-1.0)
                else:
                    nc.sync.dma_start(out=outo[t * P:(t + 1) * P, :], in_=o1[:])

            # ---------------- layer-1 edge phase + transform 2 ----------
            for t in range(NT):
                edge_tile(t, tb1, HH1, CC1, CT1, RW1, AS1, AD1,
                          hsb[:, t, :], bt1, True)
                # transform2 for this tile: g = elu_h2 @ W2p
                pT = ps2.tile([CT1, P], f32, tag="pT")
                nc.tensor.matmul(out=pT[:], lhsT=h2sb[:, t, :], rhs=idt[:],
                                 start=True, stop=True)
                tsb = tp2.tile([CT1, P], f16, tag="tsb")
                nc.vector.tensor_copy(out=tsb[:], in_=pT[:])
                p2 = ps2.tile([P, RW2], f32, tag="p2")
                nc.tensor.matmul(out=p2[:], lhsT=tsb[:], rhs=w2sb[:],
                                 start=True, stop=True)
                nc.vector.tensor_copy(out=gsb[:, t, :], in_=p2[:])
                nc.sync.dma_start(out=cc2[t * P:(t + 1) * P, 0:RW2],
                                  in_=gsb[:, t, :])

            tc.strict_bb_all_engine_barrier()
            nc.gpsimd.collective_compute(
                "AllGather", mybir.AluOpType.bypass,
                replica_groups=[list(range(NCORE))],
                ins=[cc2[:]], outs=[tb2[:]])
            tc.strict_bb_all_engine_barrier()
            for r in range(NCORE):
                nc.sync.dma_start(out=tb2[r * PC + SH:r * PC + SH + 44, :],
                                  in_=pois[:])
            tc.strict_bb_all_engine_barrier()

            # ---------------- layer-2 edge phase ------------------------
            for t in range(NT):
                edge_tile(t, tb2, 1, OUT2, OUT2, RW2, AS2, AD2,
                          gsb[:, t, :], bt2, False)

    nc.compile()
    return nc


# ==========================================================================
# top-level kernel
# ==========================================================================

def _fold_w1(W1, a_src, a_dst):
    W1r = W1.reshape(F_IN, HH1, CC1)
    ws = np.einsum("khc,hc->kh", W1r, a_src)
    wd = np.einsum("khc,hc->kh", W1r, a_dst)
    return np.concatenate([W1, ws, wd], axis=1)


def kernel(x, edge_index, W1, a_src1, a_dst1, b1, W2, a_src2, a_dst2, b2):
    from concourse.bass_utils import run_bass_kernel_spmd

    x = np.asarray(x)
    src, dst = np.asarray(edge_index[0]), np.asarray(edge_index[1])
    pos, core_id, orows, D_tbl, tile_off, CB, blobs = prep_graph(src, dst)

    nc = build_launch(D_tbl, tile_off, CB)

    # self-loop multiplicity: 1 (added loop) + natural src==dst edges
    selfc = np.bincount(dst[src == dst], minlength=N)
    lnmult = np.log1p(selfc.astype(np.float64)).astype(np.float32)

    W1p = _fold_w1(np.asarray(W1), np.asarray(a_src1),
                   np.asarray(a_dst1)).astype(np.float16)
    W2_ = np.asarray(W2)
    W2p = np.concatenate([W2_,
                          (W2_ @ np.asarray(a_src2)[0])[:, None],
                          (W2_ @ np.asarray(a_dst2)[0])[:, None]],
                         axis=1).astype(np.float16)
    ident = np.eye(P, dtype=np.float16)
    b1v = np.ascontiguousarray(b1, np.float32)
    b2v = np.ascontiguousarray(b2, np.float32)

    in_maps = []
    for r in range(NCORE):
        nodes = np.where(core_id == r)[0]
        xs = np.zeros((PC, F_IN), np.float16)
        xs[pos[nodes]] = x[nodes].astype(np.float16)
        lv = np.zeros(PC, np.float32)
        lv[pos[nodes]] = lnmult[nodes]
        in_maps.append({
            "fT": np.ascontiguousarray(xs.T),
            "Wp": W1p, "W2p": W2p, "b1t": b1v, "b2t": b2v,
            "ident": ident, "idxb": blobs[r],
            "lnm": np.ascontiguousarray(lv.reshape(NT, P).T),
        })

    res = run_bass_kernel_spmd(nc, in_maps, list(range(NCORE)))
    if res.exec_time_ns is not None:
        tr = res.instructions_and_trace
        RUNLOG.append({"layer": "fused", "exec_time_ns": res.exec_time_ns,
                       "trace": tr[1] if tr else None,
                       "profile_json": res.profile_json})

    out = np.empty((N, OUT2), np.float32)
    for r in range(NCORE):
        valid = orows[r] >= 0
        out[orows[r][valid]] = res.results[r]["out"][valid]
    return out


# revision 24
# speedup vs baseline: 1.0437x; 1.0437x over previous
"""Two-layer GAT on 8 Trainium2 NeuronCores — single launch, Bass/Tile.

v2 design
---------
* Table row order = dst-core-major: node n -> row = core(n)*12544 + pos(n).
  Each core's transform shard IS its own dst block, so per-dst "own row"
  data (h, a_src·h, a_dst·h) stays in SBUF — no reserved gather slots.
* Gather groups are row PHASES (row % 4) via elem_step=512 strided
  dma_gather (int16 idx = row//4 < 25088). Each node's phase is chosen by
  a greedy balancer so every dst segment has near-equal per-phase counts,
  cutting slot padding from ~2.1x to ~1.25x.
* Self-loop edges are folded in analytically from the SBUF-resident own
  rows (never gathered).
* x@W runs sharded (1/8 nodes per core); AllGather broadcasts the table;
  pad rows (positions 12500..12543 of each block) are poisoned with
  a_src = -60000 so their exp() contribution is exactly 0.
* Both layers run in ONE device launch; layer-2 table = elu(out1)@W2p is
  transposed+transformed on-device, AllGathered, and the SAME index blob
  drives both edge phases (identical graph layout).
* Gathers: 1024-idx calls, single_packet=False, 4 SWDGE queues (measured
  ~81 GB/s/core vs 42 GB/s for the default config).
"""
import sys
sys.path.insert(0, "/opt/trn_rl_repo")

import numpy as np

P = 128
NCORE = 8
N = 100000
SH = 12500            # real dst nodes per core
NT = 98
PC = NT * P           # 12544 padded positions per core
NV = PC * NCORE       # 100352 table rows
G = 4                 # phases
PH = NV // G          # 25088 rows per phase (int16-addressable)
PCQ = PC // G         # 3136 positions per (core, phase)
SHQ = SH // G         # 3125 real nodes per (core, phase)
F_IN = 512
HH1, CC1 = 8, 8
CT1 = HH1 * CC1       # 64
RW1 = CT1 + 2 * HH1   # 80: [h(64) | as(8) | ad(8)]
OUT2 = 40
RW2 = OUT2 + 2        # 42: [g(40) | as2 | ad2]
NEG = -60000.0
DCH = 8               # slot-columns per gather call (1024 idxs)
PAD_IDX = 3125        # core-0, k=3125 pad row (same j in every phase table)

RUNLOG = []           # filled when BASS_TRACE is set (test harness only)


# ==========================================================================
# host-side graph prep
# ==========================================================================

def assign_phases(es, ed, core_id):
    """Greedy phase assignment balancing per-dst-segment phase counts.

    Returns phase[N] int8 with exactly SH//G nodes per (core, phase).
    """
    out_deg = np.bincount(es, minlength=N)
    order = np.argsort(es, kind="stable")
    ed_s = ed[order]
    starts = np.zeros(N + 1, np.int64)
    np.cumsum(out_deg, out=starts[1:])

    cnt = np.zeros((N, G), np.int32)        # per dst, per phase in-counts
    cap = np.full((NCORE, G), SH // G, np.int32)
    phase = np.zeros(N, np.int8)
    BIG = np.int32(1 << 30)
    gidx = np.arange(G)

    proc = np.argsort(-out_deg, kind="stable")
    for s in proc:
        dsts = ed_s[starts[s]:starts[s + 1]]
        r = core_id[s]
        sc = cnt[dsts].sum(axis=0, dtype=np.int64)
        sc = np.where(cap[r] > 0, sc, BIG)
        g = int(np.argmin(sc))
        phase[s] = g
        cap[r, g] -= 1
        cnt[dsts, g] += 1

    for _ in range(3):                       # refinement sweeps
        moved = 0
        for s in proc:
            dsts = ed_s[starts[s]:starts[s + 1]]
            if len(dsts) == 0:
                continue
            r, g0 = core_id[s], phase[s]
            sc = cnt[dsts].sum(axis=0, dtype=np.int64)
            sc[g0] -= len(dsts)
            scm = np.where((cap[r] > 0) | (gidx == g0), sc, BIG)
            g = int(np.argmin(scm))
            if g != g0:
                phase[s] = g
                cap[r, g] -= 1
                cap[r, g0] += 1
                cnt[dsts, g0] -= 1
                cnt[dsts, g] += 1
                moved += 1
        if moved == 0:
            break
    return phase, cnt


def prep_graph(src, dst):
    """Build phases, positions, per-tile slot tables and idx blobs."""
    src = np.asarray(src, np.int64)
    dst = np.asarray(dst, np.int64)
    m = src != dst
    es, ed = src[m], dst[m]

    # dst -> core: deal by in-degree so every core sees the same profile
    indeg = np.bincount(ed, minlength=N)
    by_deg = np.argsort(-indeg, kind="stable")
    core_id = np.empty(N, np.int32)
    core_id[by_deg] = np.arange(N) % NCORE

    phase, cnt = assign_phases(es, ed, core_id)

    # positions: per core, per phase, in-degree-descending fill
    pos = np.empty(N, np.int64)
    orows = np.full((NCORE, PC), -1, np.int64)
    for r in range(NCORE):
        nodes = np.where(core_id == r)[0]
        nodes = nodes[np.argsort(-indeg[nodes], kind="stable")]
        for q in range(G):
            nq = nodes[phase[nodes] == q]
            pq = q * PCQ + np.arange(len(nq))
            pos[nq] = pq
            orows[r, pq] = nq
    # per-phase table row of node: j = core*PCQ + (pos % PCQ), group = pos//PCQ
    row_of = core_id.astype(np.int64) * PCQ + (pos % PCQ)

    # per-core tile max counts -> uniform D table
    D_core = np.zeros((NCORE, NT, G), np.int64)
    for r in range(NCORE):
        carr = np.zeros((PC, G), np.int32)
        nodes = np.where(core_id == r)[0]
        carr[pos[nodes]] = cnt[nodes]
        D_core[r] = carr.reshape(NT, P, G).max(axis=1)
    D_tbl = D_core.max(axis=0)                        # [NT, G]

    tile_off = []
    off = 0
    for t in range(NT):
        tile_off.append(off)
        off += 8 * int(D_tbl[t].sum())
    CB = off

    # per-core idx blobs
    idxval = row_of[es].astype(np.int16)
    gval = (pos[es] // PCQ).astype(np.int64)
    blobs = []
    dmax = int(D_tbl.max())
    for r in range(NCORE):
        mm = core_id[ed] == r
        e_pos = pos[ed[mm]]
        e_g = gval[mm]
        e_idx = idxval[mm]
        key = e_pos * G + e_g
        o = np.argsort(key, kind="stable")
        ks, vs = key[o], e_idx[o]
        cnt_pg = np.bincount(ks, minlength=PC * G)
        st = np.zeros(PC * G, np.int64)
        np.cumsum(cnt_pg[:-1], out=st[1:])
        col = np.arange(len(ks)) - st[ks]
        big = np.full((PC, G, dmax), PAD_IDX, np.int16)
        big[ks // G, ks % G, col] = vs

        blob = np.empty((P, CB), np.int16)
        for t in range(NT):
            c = tile_off[t]
            for g in range(G):
                D = int(D_tbl[t, g])
                mat = big[t * P:(t + 1) * P, g, :D]       # [128, D]
                for ck in range(0, D, DCH):
                    d = min(DCH, D - ck)
                    L = mat[:, ck:ck + d].T.ravel()       # i = col*128+part
                    W16 = L.reshape(-1, 16).T             # [16, 8d]
                    blob[:, c:c + 8 * d] = np.tile(W16, (8, 1))
                    c += 8 * d
        blobs.append(blob)

    return pos, core_id, orows, D_tbl, tile_off, CB, blobs


# ==========================================================================
# device program (single launch, both layers)
# ==========================================================================

def build_launch(D_tbl, tile_off, CB):
    from concourse import mybir
    import concourse.bass as bass
    import concourse.bacc as bacc
    import concourse.tile as tile
    import contextlib

    f16, f32, i16 = mybir.dt.float16, mybir.dt.float32, mybir.dt.int16
    AS1, AD1 = CT1, CT1 + HH1            # 64, 72
    AS2, AD2 = OUT2, OUT2 + 1            # 40, 41

    nc = bacc.Bacc("TRN2", target_bir_lowering=False, debug=False,
                   num_swdge_queues=4, num_devices=8)
    fT = nc.dram_tensor("fT", [F_IN, PC], f16, kind="ExternalInput")
    lnm = nc.dram_tensor("lnm", [P, NT], f32, kind="ExternalInput")
    Wp = nc.dram_tensor("Wp", [F_IN, RW1], f16, kind="ExternalInput")
    W2p = nc.dram_tensor("W2p", [CT1, RW2], f16, kind="ExternalInput")
    b1t = nc.dram_tensor("b1t", [CT1], f32, kind="ExternalInput")
    b2t = nc.dram_tensor("b2t", [OUT2], f32, kind="ExternalInput")
    ident = nc.dram_tensor("ident", [P, P], f16, kind="ExternalInput")
    idxb = nc.dram_tensor("idxb", [P, CB], i16, kind="ExternalInput")
    outo = nc.dram_tensor("out", [PC, OUT2], f32, kind="ExternalOutput")

    cc1 = [nc.dram_tensor(f"cc1_{q}", [PCQ, 128], f16) for q in range(G)]
    cc2 = [nc.dram_tensor(f"cc2_{q}", [PCQ, 128], f16) for q in range(G)]
    tb1 = [nc.dram_tensor(f"tb1_{q}", [NCORE * PCQ, 128], f16,
                          addr_space="Shared") for q in range(G)]
    tb2 = [nc.dram_tensor(f"tb2_{q}", [NCORE * PCQ, 128], f16,
                          addr_space="Shared") for q in range(G)]

    qn = [0]

    def nextq():
        qn[0] = (qn[0] + 1) % 4
        return qn[0]

    def bcast_load(dram_ap, w, pool, dt, tag):
        t = pool.tile([P, w], dt, tag=tag)
        ap = bass.AP(tensor=dram_ap.tensor, offset=dram_ap.offset,
                     ap=[[0, P]] + list(dram_ap.ap))
        nc.sync.dma_start(out=t[:], in_=ap)
        return t

    with tile.TileContext(nc) as tc:
        with contextlib.ExitStack() as ctx:
            singles = ctx.enter_context(tc.tile_pool(name="singles", bufs=1))
            xtp = ctx.enter_context(tc.tile_pool(name="xt", bufs=2))
            psp = ctx.enter_context(tc.tile_pool(name="ps", bufs=4, space="PSUM"))
            ps2 = ctx.enter_context(tc.tile_pool(name="ps2", bufs=2, space="PSUM"))
            gp = ctx.enter_context(tc.tile_pool(name="gp", bufs=4))
            ip = ctx.enter_context(tc.tile_pool(name="ip", bufs=3))
            ep = ctx.enter_context(tc.tile_pool(name="ep", bufs=3))
            sp = ctx.enter_context(tc.tile_pool(name="sp", bufs=4))
            mp = ctx.enter_context(tc.tile_pool(name="mp", bufs=3))
            tp2 = ctx.enter_context(tc.tile_pool(name="tp2", bufs=3))

            # ---------------- singles ----------------
            wts = []
            for kc in range(4):
                wt = singles.tile([P, RW1], f16, tag=f"w{kc}")
                nc.sync.dma_start(out=wt[:], in_=Wp[kc * P:(kc + 1) * P, :])
                wts.append(wt)
            w2sb = singles.tile([CT1, RW2], f16)
            nc.sync.dma_start(out=w2sb[:], in_=W2p[:])
            idt = singles.tile([P, P], f16)
            nc.sync.dma_start(out=idt[:], in_=ident[:])
            bt1 = bcast_load(b1t[:], CT1, singles, f32, "bt1")
            bt2 = bcast_load(b2t[:], OUT2, singles, f32, "bt2")
            # poison rows: zeros, NEG at 40:42 and 64:80
            pois = singles.tile([PCQ - SHQ, 128], f16, tag="pois")
            nc.vector.memset(pois[:], 0.0)
            nc.vector.memset(pois[:, AS2:AD2 + 1], NEG)
            nc.vector.memset(pois[:, AS1:AD1 + HH1], NEG)
            # own-row tables (SBUF-resident)
            hsb = singles.tile([P, NT, RW1], f16, tag="hsb")
            h2sb = singles.tile([P, NT, CT1], f16, tag="h2sb")
            gsb = singles.tile([P, NT, RW2], f16, tag="gsb")
            lnmt = singles.tile([P, NT], f32, tag="lnm")
            nc.sync.dma_start(out=lnmt[:], in_=lnm[:])

            def cc_write(ccs, ti, src_tile, w):
                # positions [128*ti, 128*ti+128) -> phase q at pos//PCQ
                p0 = 0
                while p0 < P:
                    gpos = ti * P + p0
                    q = gpos // PCQ
                    run = min(P - p0, (q + 1) * PCQ - gpos)
                    nc.sync.dma_start(
                        out=ccs[q][gpos - q * PCQ: gpos - q * PCQ + run, 0:w],
                        in_=src_tile[p0:p0 + run, :])
                    p0 += run

            # ---------------- transform 1: x@W1p for own block ----------
            chunks = [(c, min(512, PC - c)) for c in range(0, PC, 512)]
            for c0, cw in chunks:
                xts = []
                for kc in range(4):
                    xt = xtp.tile([P, 512], f16, tag=f"x{kc}")
                    nc.sync.dma_start(out=xt[:, :cw],
                                      in_=fT[kc * P:(kc + 1) * P, c0:c0 + cw])
                    xts.append(xt)
                for sub in range(cw // P):
                    ti = (c0 + sub * P) // P
                    pt = psp.tile([P, RW1], f32)
                    for kc in range(4):
                        nc.tensor.matmul(out=pt[:],
                                         lhsT=xts[kc][:, sub * P:(sub + 1) * P],
                                         rhs=wts[kc][:],
                                         start=(kc == 0), stop=(kc == 3))
                    nc.vector.tensor_copy(out=hsb[:, ti, :], in_=pt[:])
                    cc_write(cc1, ti, hsb[:, ti, :], RW1)

            for q in range(G):
                nc.gpsimd.collective_compute(
                    "AllGather", mybir.AluOpType.bypass,
                    replica_groups=[list(range(NCORE))],
                    ins=[cc1[q][:]], outs=[tb1[q][:]])
            tc.strict_bb_all_engine_barrier()
            for q in range(G):
                for r in range(NCORE):
                    nc.sync.dma_start(
                        out=tb1[q][r * PCQ + SHQ:r * PCQ + PCQ, :], in_=pois[:])
            tc.strict_bb_all_engine_barrier()

            # ---------------- shared edge-phase body --------------------
            def edge_tile(t, tbl, HH, CC, CT, RW, AS, AD, own, bt, is_l1):
                mybir_ = mybir
                Ds = [int(D_tbl[t, g]) for g in range(G)]
                SD = sum(Ds)
                SD1 = SD + 1                      # +1 self slot (computed)
                tcols = 8 * SD
                it = ip.tile([P, tcols], i16, tag="idx")
                nc.sync.dma_start(
                    out=it[:], in_=idxb[:, tile_off[t]:tile_off[t] + tcols])
                Gt = gp.tile([P, SD1, 128], f16, tag="G")
                c0 = 0
                ic = 0
                for g in range(G):
                    D = Ds[g]
                    nck = (D + DCH - 1) // DCH
                    base, rem = (D // nck, D % nck) if nck else (0, 0)
                    for j in range(nck):
                        d = base + (1 if j < rem else 0)
                        nc.gpsimd.dma_gather(
                            out_ap=Gt[:, c0:c0 + d, :],
                            in_ap=tbl[g][:],
                            idxs_ap=it[:, ic:ic + 8 * d],
                            num_idxs=P * d, num_idxs_reg=P * d,
                            elem_size=128,
                            single_packet=False,
                            queue_num=nextq())
                        c0 += d
                        ic += 8 * d
                # self slot: own row (h | as | ad)
                nc.vector.tensor_copy(out=Gt[:, SD, 0:RW], in_=own[:])

                # e = leaky(as_src + ad_own)   [P, SD1, HH] f32
                et = ep.tile([P, SD1, HH], f32, tag="e")
                nc.vector.tensor_tensor(
                    out=et[:],
                    in0=Gt[:, :, AS:AS + HH],
                    in1=own[:, AD:AD + HH].unsqueeze(1)
                        .broadcast_to([P, SD1, HH]),
                    op=mybir_.AluOpType.add)
                ef = et[:].rearrange("p d h -> p (d h)")
                nc.scalar.activation(out=ef, in_=ef,
                                     func=mybir_.ActivationFunctionType.Prelu,
                                     alpha=0.2)
                # self-loop multiplicity: e_self += ln(mult)
                nc.vector.tensor_scalar_add(out=et[:, SD, :],
                                            in0=et[:, SD, :],
                                            scalar1=lnmt[:, t:t + 1])
                # -max, subtract, exp
                nmt = sp.tile([P, HH], f32, tag="nm")
                nc.vector.reduce_max(out=nmt[:],
                                     in_=et[:].rearrange("p d h -> p h d"),
                                     axis=mybir_.AxisListType.X, negate=True)
                nc.vector.tensor_tensor(
                    out=et[:], in0=et[:],
                    in1=nmt[:].unsqueeze(1).broadcast_to([P, SD1, HH]),
                    op=mybir_.AluOpType.add)
                ext = ep.tile([P, SD1, HH], f16, tag="ex")
                nc.scalar.activation(out=ext[:].rearrange("p d h -> p (d h)"),
                                     in_=ef,
                                     func=mybir_.ActivationFunctionType.Exp)

                # denom + reciprocal
                dent = sp.tile([P, HH], f32, tag="den")
                nc.vector.reduce_sum(out=dent[:],
                                     in_=ext[:].rearrange("p d h -> p h d"),
                                     axis=mybir_.AxisListType.X)
                rdt = sp.tile([P, HH], f32, tag="rd")
                nc.vector.reciprocal(out=rdt[:], in_=dent[:])

                # weighted message sum (f16 products, f32 accumulate)
                mg = mp.tile([P, SD1, CT], f16, tag="msg")
                nc.vector.tensor_tensor(
                    out=mg[:].rearrange("p d (h c) -> p d h c", h=HH),
                    in0=Gt[:, :, 0:CT].rearrange("p d (h c) -> p d h c", h=HH),
                    in1=ext[:].unsqueeze(3).broadcast_to([P, SD1, HH, CC]),
                    op=mybir_.AluOpType.mult)
                ort = sp.tile([P, CT], f32, tag="or")
                nc.vector.reduce_sum(out=ort[:],
                                     in_=mg[:].rearrange("p d f -> p f d"),
                                     axis=mybir_.AxisListType.X)

                # normalize + bias
                o1 = sp.tile([P, CT], f32, tag="o1")
                nc.vector.tensor_tensor(
                    out=o1[:].rearrange("p (h c) -> p h c", h=HH),
                    in0=ort[:].rearrange("p (h c) -> p h c", h=HH),
                    in1=rdt[:].unsqueeze(2).broadcast_to([P, HH, CC]),
                    op=mybir_.AluOpType.mult)
                nc.vector.tensor_add(out=o1[:], in0=o1[:], in1=bt[:])

                if is_l1:
                    # elu(x) = relu(x) + exp(min(x,0)) - 1
                    t1 = sp.tile([P, CT], f32, tag="t1")
                    nc.vector.tensor_scalar_min(out=t1[:], in0=o1[:], scalar1=0.0)
                    nc.scalar.activation(out=t1[:], in_=t1[:],
                                         func=mybir_.ActivationFunctionType.Exp)
                    t2 = sp.tile([P, CT], f32, tag="t2")
                    nc.scalar.activation(out=t2[:], in_=o1[:],
                                         func=mybir_.ActivationFunctionType.Relu)
                    nc.vector.tensor_add(out=t1[:], in0=t1[:], in1=t2[:])
                    nc.vector.tensor_scalar_add(out=h2sb[:, t, :], in0=t1[:],
                                                scalar1=-1.0)
                else:
                    nc.sync.dma_start(out=outo[t * P:(t + 1) * P, :], in_=o1[:])

            # ---------------- layer-1 edge phase + transform 2 ----------
            for t in range(NT):
                edge_tile(t, tb1, HH1, CC1, CT1, RW1, AS1, AD1,
                          hsb[:, t, :], bt1, True)
                # transform2 for this tile: g = elu_h2 @ W2p
                pT = ps2.tile([CT1, P], f32, tag="pT")
                nc.tensor.matmul(out=pT[:], lhsT=h2sb[:, t, :], rhs=idt[:],
                                 start=True, stop=True)
                tsb = tp2.tile([CT1, P], f16, tag="tsb")
                nc.vector.tensor_copy(out=tsb[:], in_=pT[:])
                p2 = ps2.tile([P, RW2], f32, tag="p2")
                nc.tensor.matmul(out=p2[:], lhsT=tsb[:], rhs=w2sb[:],
                                 start=True, stop=True)
                nc.vector.tensor_copy(out=gsb[:, t, :], in_=p2[:])
                cc_write(cc2, t, gsb[:, t, :], RW2)

            tc.strict_bb_all_engine_barrier()
            for q in range(G):
                nc.gpsimd.collective_compute(
                    "AllGather", mybir.AluOpType.bypass,
                    replica_groups=[list(range(NCORE))],
                    ins=[cc2[q][:]], outs=[tb2[q][:]])
            tc.strict_bb_all_engine_barrier()
            for q in range(G):
                for r in range(NCORE):
                    nc.sync.dma_start(
                        out=tb2[q][r * PCQ + SHQ:r * PCQ + PCQ, :], in_=pois[:])
            tc.strict_bb_all_engine_barrier()

            # ---------------- layer-2 edge phase ------------------------
            for t in range(NT):
                edge_tile(t, tb2, 1, OUT2, OUT2, RW2, AS2, AD2,
                          gsb[:, t, :], bt2, False)

    nc.compile()
    return nc


# ==========================================================================
# top-level kernel
# ==========================================================================

def _fold_w1(W1, a_src, a_dst):
    W1r = W1.reshape(F_IN, HH1, CC1)
    ws = np.einsum("khc,hc->kh", W1r, a_src)
    wd = np.einsum("khc,hc->kh", W1r, a_dst)
    return np.concatenate([W1, ws, wd], axis=1)


def kernel(x, edge_index, W1, a_src1, a_dst1, b1, W2, a_src2, a_dst2, b2):
    from concourse.bass_utils import run_bass_kernel_spmd

    x = np.asarray(x)
    src, dst = np.asarray(edge_index[0]), np.asarray(edge_index[1])
    pos, core_id, orows, D_tbl, tile_off, CB, blobs = prep_graph(src, dst)

    nc = build_launch(D_tbl, tile_off, CB)

    # self-loop multiplicity: 1 (added loop) + natural src==dst edges
    selfc = np.bincount(dst[src == dst], minlength=N)
    lnmult = np.log1p(selfc.astype(np.float64)).astype(np.float32)

    W1p = _fold_w1(np.asarray(W1), np.asarray(a_src1),
                   np.asarray(a_dst1)).astype(np.float16)
    W2_ = np.asarray(W2)
    W2p = np.concatenate([W2_,
                          (W2_ @ np.asarray(a_src2)[0])[:, None],
                          (W2_ @ np.asarray(a_dst2)[0])[:, None]],
                         axis=1).astype(np.float16)
    ident = np.eye(P, dtype=np.float16)
    b1v = np.ascontiguousarray(b1, np.float32)
    b2v = np.ascontiguousarray(b2, np.float32)

    in_maps = []
    for r in range(NCORE):
        nodes = np.where(core_id == r)[0]
        xs = np.zeros((PC, F_IN), np.float16)
        xs[pos[nodes]] = x[nodes].astype(np.float16)
        lv = np.zeros(PC, np.float32)
        lv[pos[nodes]] = lnmult[nodes]
        in_maps.append({
            "fT": np.ascontiguousarray(xs.T),
            "Wp": W1p, "W2p": W2p, "b1t": b1v, "b2t": b2v,
            "ident": ident, "idxb": blobs[r],
            "lnm": np.ascontiguousarray(lv.reshape(NT, P).T),
        })

    res = run_bass_kernel_spmd(nc, in_maps, list(range(NCORE)))
    if res.exec_time_ns is not None:
        tr = res.instructions_and_trace
        RUNLOG.append({"layer": "fused", "exec_time_ns": res.exec_time_ns,
                       "trace": tr[1] if tr else None,
                       "profile_json": res.profile_json})

    out = np.empty((N, OUT2), np.float32)
    for r in range(NCORE):
        valid = orows[r] >= 0
        out[orows[r][valid]] = res.results[r]["out"][valid]
    return out


# revision 26
# speedup vs baseline: 1.0737x; 1.0287x over previous
"""Two-layer GAT on 8 Trainium2 NeuronCores — single launch, Bass/Tile.

Design (baseline 12.18 ms -> ~5.6 ms)
-------------------------------------
* Destination nodes are dealt to the 8 cores by in-degree (uniform
  profiles); within a core, positions are phase-major: pos = q*3136 + k,
  q = the node's gather phase, k = per-phase in-degree rank. Each core's
  transform shard IS its own dst block, so per-dst "own row" data
  (h | a_src.h | a_dst.h) stays in SBUF and self-loop edges are folded in
  analytically (never gathered); natural src==dst duplicates are handled
  by adding ln(multiplicity) to the self score before exp.
* The node feature table is split into G=4 per-phase CONTIGUOUS tables of
  25088 x 256B rows (int16-indexable; measured 3.16 ns/idx vs 3.65 for a
  strided layout). Each node's phase is chosen by a greedy balancer so
  every dst segment has near-equal per-phase counts, cutting gather slot
  padding from 2.11x to ~1.55x.
* x@W runs sharded (1/8 nodes per core); 4 AllGathers broadcast the
  table; pad rows are poisoned with a_src = -60000 so exp() -> 0 exactly.
* Both layers run in ONE device launch; the layer-2 table
  (elu(out1) @ W2p) is built on-device via a TensorE identity-transpose,
  AllGathered, and the SAME index blob drives both edge phases.
* Gathers: 1024-idx dma_gather calls, single_packet=False, 4 SWDGE
  queues round-robin (measured 81 GB/s/core vs 42 GB/s default config).
* Edge-phase softmax: scores f32 [P, slots, heads] with contiguous inner
  dim, Prelu+Exp on the Scalar engine, f16 weighted messages with f32
  reduction; max-subtraction keeps exp in f16 range.
"""
import sys
sys.path.insert(0, "/opt/trn_rl_repo")

import numpy as np

P = 128
NCORE = 8
N = 100000
SH = 12500            # real dst nodes per core
NT = 98
PC = NT * P           # 12544 padded positions per core
NV = PC * NCORE       # 100352 table rows
G = 4                 # phases
PH = NV // G          # 25088 rows per phase (int16-addressable)
PCQ = PC // G         # 3136 positions per (core, phase)
SHQ = SH // G         # 3125 real nodes per (core, phase)
F_IN = 512
HH1, CC1 = 8, 8
CT1 = HH1 * CC1       # 64
RW1 = CT1 + 2 * HH1   # 80: [h(64) | as(8) | ad(8)]
OUT2 = 40
RW2 = OUT2 + 2        # 42: [g(40) | as2 | ad2]
NEG = -60000.0
DCH = 8               # slot-columns per gather call (1024 idxs)
PAD_IDX = 3125        # core-0, k=3125 pad row (same j in every phase table)

RUNLOG = []           # filled when BASS_TRACE is set (test harness only)


# ==========================================================================
# host-side graph prep
# ==========================================================================

def assign_phases(es, ed, core_id):
    """Greedy phase assignment balancing per-dst-segment phase counts.

    Returns phase[N] int8 with exactly SH//G nodes per (core, phase).
    """
    out_deg = np.bincount(es, minlength=N)
    order = np.argsort(es, kind="stable")
    ed_s = ed[order]
    starts = np.zeros(N + 1, np.int64)
    np.cumsum(out_deg, out=starts[1:])

    cnt = np.zeros((N, G), np.int32)        # per dst, per phase in-counts
    cap = np.full((NCORE, G), SH // G, np.int32)
    phase = np.zeros(N, np.int8)
    BIG = np.int32(1 << 30)
    gidx = np.arange(G)

    proc = np.argsort(-out_deg, kind="stable")
    for s in proc:
        dsts = ed_s[starts[s]:starts[s + 1]]
        r = core_id[s]
        sc = cnt[dsts].sum(axis=0, dtype=np.int64)
        sc = np.where(cap[r] > 0, sc, BIG)
        g = int(np.argmin(sc))
        phase[s] = g
        cap[r, g] -= 1
        cnt[dsts, g] += 1

    for _ in range(3):                       # refinement sweeps
        moved = 0
        for s in proc:
            dsts = ed_s[starts[s]:starts[s + 1]]
            if len(dsts) == 0:
                continue
            r, g0 = core_id[s], phase[s]
            sc = cnt[dsts].sum(axis=0, dtype=np.int64)
            sc[g0] -= len(dsts)
            scm = np.where((cap[r] > 0) | (gidx == g0), sc, BIG)
            g = int(np.argmin(scm))
            if g != g0:
                phase[s] = g
                cap[r, g] -= 1
                cap[r, g0] += 1
                cnt[dsts, g0] -= 1
                cnt[dsts, g] += 1
                moved += 1
        if moved == 0:
            break
    return phase, cnt


def prep_graph(src, dst):
    """Build phases, positions, per-tile slot tables and idx blobs."""
    src = np.asarray(src, np.int64)
    dst = np.asarray(dst, np.int64)
    m = src != dst
    es, ed = src[m], dst[m]

    # dst -> core: deal by in-degree so every core sees the same profile
    indeg = np.bincount(ed, minlength=N)
    by_deg = np.argsort(-indeg, kind="stable")
    core_id = np.empty(N, np.int32)
    core_id[by_deg] = np.arange(N) % NCORE

    phase, cnt = assign_phases(es, ed, core_id)

    # positions: per core, per phase, in-degree-descending fill
    pos = np.empty(N, np.int64)
    orows = np.full((NCORE, PC), -1, np.int64)
    for r in range(NCORE):
        nodes = np.where(core_id == r)[0]
        nodes = nodes[np.argsort(-indeg[nodes], kind="stable")]
        for q in range(G):
            nq = nodes[phase[nodes] == q]
            pq = q * PCQ + np.arange(len(nq))
            pos[nq] = pq
            orows[r, pq] = nq
    # per-phase table row of node: j = core*PCQ + (pos % PCQ), group = pos//PCQ
    row_of = core_id.astype(np.int64) * PCQ + (pos % PCQ)

    # per-core tile max counts -> uniform D table
    D_core = np.zeros((NCORE, NT, G), np.int64)
    for r in range(NCORE):
        carr = np.zeros((PC, G), np.int32)
        nodes = np.where(core_id == r)[0]
        carr[pos[nodes]] = cnt[nodes]
        D_core[r] = carr.reshape(NT, P, G).max(axis=1)
    D_tbl = D_core.max(axis=0)                        # [NT, G]

    tile_off = []
    off = 0
    for t in range(NT):
        tile_off.append(off)
        off += 8 * int(D_tbl[t].sum())
    CB = off

    # per-core idx blobs
    idxval = row_of[es].astype(np.int16)
    gval = (pos[es] // PCQ).astype(np.int64)
    blobs = []
    dmax = int(D_tbl.max())
    for r in range(NCORE):
        mm = core_id[ed] == r
        e_pos = pos[ed[mm]]
        e_g = gval[mm]
        e_idx = idxval[mm]
        key = e_pos * G + e_g
        o = np.argsort(key, kind="stable")
        ks, vs = key[o], e_idx[o]
        cnt_pg = np.bincount(ks, minlength=PC * G)
        st = np.zeros(PC * G, np.int64)
        np.cumsum(cnt_pg[:-1], out=st[1:])
        col = np.arange(len(ks)) - st[ks]
        big = np.full((PC, G, dmax), PAD_IDX, np.int16)
        big[ks // G, ks % G, col] = vs

        blob = np.empty((P, CB), np.int16)
        for t in range(NT):
            c = tile_off[t]
            for g in range(G):
                D = int(D_tbl[t, g])
                mat = big[t * P:(t + 1) * P, g, :D]       # [128, D]
                for ck in range(0, D, DCH):
                    d = min(DCH, D - ck)
                    L = mat[:, ck:ck + d].T.ravel()       # i = col*128+part
                    W16 = L.reshape(-1, 16).T             # [16, 8d]
                    blob[:, c:c + 8 * d] = np.tile(W16, (8, 1))
                    c += 8 * d
        blobs.append(blob)

    return pos, core_id, orows, D_tbl, tile_off, CB, blobs


# ==========================================================================
# device program (single launch, both layers)
# ==========================================================================

def build_launch(D_tbl, tile_off, CB):
    from concourse import mybir
    import concourse.bass as bass
    import concourse.bacc as bacc
    import concourse.tile as tile
    import contextlib

    f16, f32, i16 = mybir.dt.float16, mybir.dt.float32, mybir.dt.int16
    AS1, AD1 = CT1, CT1 + HH1            # 64, 72
    AS2, AD2 = OUT2, OUT2 + 1            # 40, 41

    nc = bacc.Bacc("TRN2", target_bir_lowering=False, debug=False,
                   num_swdge_queues=4, num_devices=8)
    fT = nc.dram_tensor("fT", [F_IN, PC], f16, kind="ExternalInput")
    lnm = nc.dram_tensor("lnm", [P, NT], f32, kind="ExternalInput")
    Wp = nc.dram_tensor("Wp", [F_IN, RW1], f16, kind="ExternalInput")
    W2p = nc.dram_tensor("W2p", [CT1, RW2], f16, kind="ExternalInput")
    b1t = nc.dram_tensor("b1t", [CT1], f32, kind="ExternalInput")
    b2t = nc.dram_tensor("b2t", [OUT2], f32, kind="ExternalInput")
    ident = nc.dram_tensor("ident", [P, P], f16, kind="ExternalInput")
    idxb = nc.dram_tensor("idxb", [P, CB], i16, kind="ExternalInput")
    outo = nc.dram_tensor("out", [PC, OUT2], f32, kind="ExternalOutput")

    cc1 = [nc.dram_tensor(f"cc1_{q}", [PCQ, 128], f16) for q in range(G)]
    cc2 = [nc.dram_tensor(f"cc2_{q}", [PCQ, 128], f16) for q in range(G)]
    tb1 = [nc.dram_tensor(f"tb1_{q}", [NCORE * PCQ, 128], f16,
                          addr_space="Shared") for q in range(G)]
    tb2 = [nc.dram_tensor(f"tb2_{q}", [NCORE * PCQ, 128], f16,
                          addr_space="Shared") for q in range(G)]

    qn = [0]

    def nextq():
        qn[0] = (qn[0] + 1) % 4
        return qn[0]

    def bcast_load(dram_ap, w, pool, dt, tag):
        t = pool.tile([P, w], dt, tag=tag)
        ap = bass.AP(tensor=dram_ap.tensor, offset=dram_ap.offset,
                     ap=[[0, P]] + list(dram_ap.ap))
        nc.sync.dma_start(out=t[:], in_=ap)
        return t

    with tile.TileContext(nc) as tc:
        with contextlib.ExitStack() as ctx:
            singles = ctx.enter_context(tc.tile_pool(name="singles", bufs=1))
            xtp = ctx.enter_context(tc.tile_pool(name="xt", bufs=2))
            psp = ctx.enter_context(tc.tile_pool(name="ps", bufs=4, space="PSUM"))
            ps2 = ctx.enter_context(tc.tile_pool(name="ps2", bufs=2, space="PSUM"))
            gp = ctx.enter_context(tc.tile_pool(name="gp", bufs=3))
            ip = ctx.enter_context(tc.tile_pool(name="ip", bufs=3))
            ep = ctx.enter_context(tc.tile_pool(name="ep", bufs=3))
            sp = ctx.enter_context(tc.tile_pool(name="sp", bufs=4))
            mp = ctx.enter_context(tc.tile_pool(name="mp", bufs=3))
            tp2 = ctx.enter_context(tc.tile_pool(name="tp2", bufs=3))

            # ---------------- singles ----------------
            wts = []
            for kc in range(4):
                wt = singles.tile([P, RW1], f16, tag=f"w{kc}")
                nc.sync.dma_start(out=wt[:], in_=Wp[kc * P:(kc + 1) * P, :])
                wts.append(wt)
            w2sb = singles.tile([CT1, RW2], f16)
            nc.sync.dma_start(out=w2sb[:], in_=W2p[:])
            idt = singles.tile([P, P], f16)
            nc.sync.dma_start(out=idt[:], in_=ident[:])
            bt1 = bcast_load(b1t[:], CT1, singles, f32, "bt1")
            bt2 = bcast_load(b2t[:], OUT2, singles, f32, "bt2")
            # poison rows: zeros, NEG at 40:42 and 64:80
            pois = singles.tile([PCQ - SHQ, 128], f16, tag="pois")
            nc.vector.memset(pois[:], 0.0)
            nc.vector.memset(pois[:, AS2:AD2 + 1], NEG)
            nc.vector.memset(pois[:, AS1:AD1 + HH1], NEG)
            # own-row tables (SBUF-resident)
            hsb = singles.tile([P, NT, RW1], f16, tag="hsb")
            h2sb = singles.tile([P, NT, CT1], f16, tag="h2sb")
            gsb = singles.tile([P, NT, RW2], f16, tag="gsb")
            lnmt = singles.tile([P, NT], f32, tag="lnm")
            nc.sync.dma_start(out=lnmt[:], in_=lnm[:])

            def cc_write(ccs, ti, src_tile, w):
                # positions [128*ti, 128*ti+128) -> phase q at pos//PCQ
                p0 = 0
                while p0 < P:
                    gpos = ti * P + p0
                    q = gpos // PCQ
                    run = min(P - p0, (q + 1) * PCQ - gpos)
                    nc.sync.dma_start(
                        out=ccs[q][gpos - q * PCQ: gpos - q * PCQ + run, 0:w],
                        in_=src_tile[p0:p0 + run, :])
                    p0 += run

            # ---------------- transform 1: x@W1p for own block ----------
            chunks = [(c, min(512, PC - c)) for c in range(0, PC, 512)]
            for c0, cw in chunks:
                xts = []
                for kc in range(4):
                    xt = xtp.tile([P, 512], f16, tag=f"x{kc}")
                    nc.sync.dma_start(out=xt[:, :cw],
                                      in_=fT[kc * P:(kc + 1) * P, c0:c0 + cw])
                    xts.append(xt)
                for sub in range(cw // P):
                    ti = (c0 + sub * P) // P
                    pt = psp.tile([P, RW1], f32)
                    for kc in range(4):
                        nc.tensor.matmul(out=pt[:],
                                         lhsT=xts[kc][:, sub * P:(sub + 1) * P],
                                         rhs=wts[kc][:],
                                         start=(kc == 0), stop=(kc == 3))
                    nc.vector.tensor_copy(out=hsb[:, ti, :], in_=pt[:])
                    cc_write(cc1, ti, hsb[:, ti, :], RW1)

            for q in range(G):
                nc.gpsimd.collective_compute(
                    "AllGather", mybir.AluOpType.bypass,
                    replica_groups=[list(range(NCORE))],
                    ins=[cc1[q][:]], outs=[tb1[q][:]])
            tc.strict_bb_all_engine_barrier()
            for q in range(G):
                for r in range(NCORE):
                    nc.sync.dma_start(
                        out=tb1[q][r * PCQ + SHQ:r * PCQ + PCQ, :], in_=pois[:])
            tc.strict_bb_all_engine_barrier()

            # ---------------- shared edge-phase body --------------------
            def edge_tile(t, tbl, HH, CC, CT, RW, AS, AD, own, bt, is_l1):
                mybir_ = mybir
                Ds = [int(D_tbl[t, g]) for g in range(G)]
                SD = sum(Ds)
                SD1 = SD + 1                      # +1 self slot (computed)
                tcols = 8 * SD
                it = ip.tile([P, tcols], i16, tag="idx")
                nc.sync.dma_start(
                    out=it[:], in_=idxb[:, tile_off[t]:tile_off[t] + tcols])
                Gt = gp.tile([P, SD1, 128], f16, tag="G")
                c0 = 0
                ic = 0
                for g in range(G):
                    D = Ds[g]
                    nck = (D + DCH - 1) // DCH
                    base, rem = (D // nck, D % nck) if nck else (0, 0)
                    for j in range(nck):
                        d = base + (1 if j < rem else 0)
                        nc.gpsimd.dma_gather(
                            out_ap=Gt[:, c0:c0 + d, :],
                            in_ap=tbl[g][:],
                            idxs_ap=it[:, ic:ic + 8 * d],
                            num_idxs=P * d, num_idxs_reg=P * d,
                            elem_size=128,
                            single_packet=False,
                            queue_num=nextq())
                        c0 += d
                        ic += 8 * d
                # self slot: own row (h | as | ad)
                nc.vector.tensor_copy(out=Gt[:, SD, 0:RW], in_=own[:])

                # e = leaky(as_src + ad_own)   [P, SD1, HH] f32
                et = ep.tile([P, SD1, HH], f32, tag="e")
                nc.vector.tensor_tensor(
                    out=et[:],
                    in0=Gt[:, :, AS:AS + HH],
                    in1=own[:, AD:AD + HH].unsqueeze(1)
                        .broadcast_to([P, SD1, HH]),
                    op=mybir_.AluOpType.add)
                ef = et[:].rearrange("p d h -> p (d h)")
                nc.scalar.activation(out=ef, in_=ef,
                                     func=mybir_.ActivationFunctionType.Prelu,
                                     alpha=0.2)
                # self-loop multiplicity: e_self += ln(mult)
                nc.vector.tensor_scalar_add(out=et[:, SD, :],
                                            in0=et[:, SD, :],
                                            scalar1=lnmt[:, t:t + 1])
                # -max, subtract, exp
                nmt = sp.tile([P, HH], f32, tag="nm")
                nc.vector.reduce_max(out=nmt[:],
                                     in_=et[:].rearrange("p d h -> p h d"),
                                     axis=mybir_.AxisListType.X, negate=True)
                nc.vector.tensor_tensor(
                    out=et[:], in0=et[:],
                    in1=nmt[:].unsqueeze(1).broadcast_to([P, SD1, HH]),
                    op=mybir_.AluOpType.add)
                ext = ep.tile([P, SD1, HH], f16, tag="ex")
                nc.scalar.activation(out=ext[:].rearrange("p d h -> p (d h)"),
                                     in_=ef,
                                     func=mybir_.ActivationFunctionType.Exp)

                # denom + reciprocal
                dent = sp.tile([P, HH], f32, tag="den")
                nc.vector.reduce_sum(out=dent[:],
                                     in_=ext[:].rearrange("p d h -> p h d"),
                                     axis=mybir_.AxisListType.X)
                rdt = sp.tile([P, HH], f32, tag="rd")
                nc.vector.reciprocal(out=rdt[:], in_=dent[:])

                # weighted message sum (f16 products, f32 accumulate)
                mg = mp.tile([P, SD1, CT], f16, tag="msg")
                nc.vector.tensor_tensor(
                    out=mg[:].rearrange("p d (h c) -> p d h c", h=HH),
                    in0=Gt[:, :, 0:CT].rearrange("p d (h c) -> p d h c", h=HH),
                    in1=ext[:].unsqueeze(3).broadcast_to([P, SD1, HH, CC]),
                    op=mybir_.AluOpType.mult)
                ort = sp.tile([P, CT], f32, tag="or")
                nc.vector.reduce_sum(out=ort[:],
                                     in_=mg[:].rearrange("p d f -> p f d"),
                                     axis=mybir_.AxisListType.X)

                # normalize + bias
                o1 = sp.tile([P, CT], f32, tag="o1")
                nc.vector.tensor_tensor(
                    out=o1[:].rearrange("p (h c) -> p h c", h=HH),
                    in0=ort[:].rearrange("p (h c) -> p h c", h=HH),
                    in1=rdt[:].unsqueeze(2).broadcast_to([P, HH, CC]),
                    op=mybir_.AluOpType.mult)
                nc.vector.tensor_add(out=o1[:], in0=o1[:], in1=bt[:])

                if is_l1:
                    # elu(x) = relu(x) + exp(min(x,0)) - 1
                    t1 = sp.tile([P, CT], f32, tag="t1")
                    nc.vector.tensor_scalar_min(out=t1[:], in0=o1[:], scalar1=0.0)
                    nc.scalar.activation(out=t1[:], in_=t1[:],
                                         func=mybir_.ActivationFunctionType.Exp)
                    t2 = sp.tile([P, CT], f32, tag="t2")
                    nc.scalar.activation(out=t2[:], in_=o1[:],
                                         func=mybir_.ActivationFunctionType.Relu)
                    nc.vector.tensor_add(out=t1[:], in0=t1[:], in1=t2[:])
                    nc.vector.tensor_scalar_add(out=h2sb[:, t, :], in0=t1[:],
                                                scalar1=-1.0)
                else:
                    nc.sync.dma_start(out=outo[t * P:(t + 1) * P, :], in_=o1[:])

            # ---------------- layer-1 edge phase + transform 2 ----------
            for t in range(NT):
                edge_tile(t, tb1, HH1, CC1, CT1, RW1, AS1, AD1,
                          hsb[:, t, :], bt1, True)
                # transform2 for this tile: g = elu_h2 @ W2p
                pT = ps2.tile([CT1, P], f32, tag="pT")
                nc.tensor.matmul(out=pT[:], lhsT=h2sb[:, t, :], rhs=idt[:],
                                 start=True, stop=True)
                tsb = tp2.tile([CT1, P], f16, tag="tsb")
                nc.vector.tensor_copy(out=tsb[:], in_=pT[:])
                p2 = ps2.tile([P, RW2], f32, tag="p2")
                nc.tensor.matmul(out=p2[:], lhsT=tsb[:], rhs=w2sb[:],
                                 start=True, stop=True)
                nc.vector.tensor_copy(out=gsb[:, t, :], in_=p2[:])
                cc_write(cc2, t, gsb[:, t, :], RW2)

            tc.strict_bb_all_engine_barrier()
            for q in range(G):
                nc.gpsimd.collective_compute(
                    "AllGather", mybir.AluOpType.bypass,
                    replica_groups=[list(range(NCORE))],
                    ins=[cc2[q][:]], outs=[tb2[q][:]])
            tc.strict_bb_all_engine_barrier()
            for q in range(G):
                for r in range(NCORE):
                    nc.sync.dma_start(
                        out=tb2[q][r * PCQ + SHQ:r * PCQ + PCQ, :], in_=pois[:])
            tc.strict_bb_all_engine_barrier()

            # ---------------- layer-2 edge phase ------------------------
            for t in range(NT):
                edge_tile(t, tb2, 1, OUT2, OUT2, RW2, AS2, AD2,
                          gsb[:, t, :], bt2, False)

    nc.compile()
    return nc


# ==========================================================================
# top-level kernel
# ==========================================================================

def _fold_w1(W1, a_src, a_dst):
    W1r = W1.reshape(F_IN, HH1, CC1)
    ws = np.einsum("khc,hc->kh", W1r, a_src)
    wd = np.einsum("khc,hc->kh", W1r, a_dst)
    return np.concatenate([W1, ws, wd], axis=1)


def kernel(x, edge_index, W1, a_src1, a_dst1, b1, W2, a_src2, a_dst2, b2):
    from concourse.bass_utils import run_bass_kernel_spmd

    x = np.asarray(x)
    src, dst = np.asarray(edge_index[0]), np.asarray(edge_index[1])
    pos, core_id, orows, D_tbl, tile_off, CB, blobs = prep_graph(src, dst)

    nc = build_launch(D_tbl, tile_off, CB)

    # self-loop multiplicity: 1 (added loop) + natural src==dst edges
    selfc = np.bincount(dst[src == dst], minlength=N)
    lnmult = np.log1p(selfc.astype(np.float64)).astype(np.float32)

    W1p = _fold_w1(np.asarray(W1), np.asarray(a_src1),
                   np.asarray(a_dst1)).astype(np.float16)
    W2_ = np.asarray(W2)
    W2p = np.concatenate([W2_,
                          (W2_ @ np.asarray(a_src2)[0])[:, None],
                          (W2_ @ np.asarray(a_dst2)[0])[:, None]],
                         axis=1).astype(np.float16)
    ident = np.eye(P, dtype=np.float16)
    b1v = np.ascontiguousarray(b1, np.float32)
    b2v = np.ascontiguousarray(b2, np.float32)

    in_maps = []
    for r in range(NCORE):
        nodes = np.where(core_id == r)[0]
        xs = np.zeros((PC, F_IN), np.float16)
        xs[pos[nodes]] = x[nodes].astype(np.float16)
        lv = np.zeros(PC, np.float32)
        lv[pos[nodes]] = lnmult[nodes]
        in_maps.append({
            "fT": np.ascontiguousarray(xs.T),
            "Wp": W1p, "W2p": W2p, "b1t": b1v, "b2t": b2v,
            "ident": ident, "idxb": blobs[r],
            "lnm": np.ascontiguousarray(lv.reshape(NT, P).T),
        })

    res = run_bass_kernel_spmd(nc, in_maps, list(range(NCORE)))
    if res.exec_time_ns is not None:
        tr = res.instructions_and_trace
        RUNLOG.append({"layer": "fused", "exec_time_ns": res.exec_time_ns,
                       "trace": tr[1] if tr else None,
                       "profile_json": res.profile_json})

    out = np.empty((N, OUT2), np.float32)
    for r in range(NCORE):
        valid = orows[r] >= 0
        out[orows[r][valid]] = res.results[r]["out"][valid]
    return out
